# revision 6
# baseline (speedup 1.0000x reference)
# Trainium2 Bass kernel for nn_MHA_18657337934739
#
# MHA: qkv = x@Wqkv + b; q,k = rope(q),rope(k); softmax(q k^T / 8) @ v; proj.
# Shapes: B=4, T=2048, C=1024, H=16 heads, D=64.
#
# Sharding: 8 cores = (4 batches) x (2 head-groups of 8 heads).  Each core
# computes its batch's attention for its 8 heads plus the partial output
# projection (contraction over its 512 local channels).  Host sums the two
# partials per batch (tensor-parallel unshard) and transposes back.
#
# On-core dataflow (all matmul inputs bf16, PSUM accumulation f32):
#   qk_T[c', t] = Wqk_loc^T x^T   (channels on partitions -> RoPE via
#                                  partition-pair swap DMA + cos/sin tables)
#   v[t, d]     = x Wv_loc        (tokens on partitions; +ones column)
#   S_T[s, q]   = K_rot^T Q_rot   (row-tiled pairs: two K=64 matmuls share
#                                  the PE array via tile_position rows 0/64)
#   P = exp(S_T / 8)              (ScalarE, no max-subtraction: |S|<~4)
#   O'[d+1, q]  = [V|1]^T P       (M=65: row 64 = softmax denominator)
#   attnT       = O'[0:64]/denom  (recip + partition-broadcast via DRAM)
#   out_T       = Wproj_loc^T attnT + b  (partial; host sums group pairs)

import numpy as np
import ml_dtypes

import concourse.bass as bass
import concourse.tile as tile
from concourse import bacc, mybir
from concourse.bass_utils import run_bass_kernel_spmd

BF16 = mybir.dt.bfloat16
F32 = mybir.dt.float32

B, T, C = 4, 2048, 1024
H, D = 16, 64
ROPE_BASE = 10000.0
SCALE = 1.0 / 8.0  # 1/sqrt(D)

NCORES = 8
LH = 8          # local heads per core
PAIRS = LH // 2  # 4
CS = C // 128    # 8 contraction subtiles
TT = T // 128    # 16 token tiles
CH = 512         # q-chunk width
NCHUNK = T // CH  # 4
VW = D + 1       # 65: V plus ones column


def _bf16(a):
    return np.ascontiguousarray(a).astype(ml_dtypes.bfloat16)


def _f32(a):
    return np.ascontiguousarray(a).astype(np.float32)


def build_program():
    nc = bacc.Bacc("TRN2", target_bir_lowering=False, debug=False)

    xT = nc.dram_tensor("xT", [128, CS, T], BF16, kind="ExternalInput")
    wqk = nc.dram_tensor("wqk", [128, CS, 1024], BF16, kind="ExternalInput")
    wv = nc.dram_tensor("wv", [128, CS, LH * D], BF16, kind="ExternalInput")
    wpo = nc.dram_tensor("wpo", [128, PAIRS, 1024], BF16, kind="ExternalInput")
    bqk = nc.dram_tensor("bqk", [128, 8], F32, kind="ExternalInput")
    bvbc = nc.dram_tensor("bvbc", [128, LH * D], F32, kind="ExternalInput")
    bpo = nc.dram_tensor("bpo", [128, 8], F32, kind="ExternalInput")
    cosb = nc.dram_tensor("cosb", [128, T], BF16, kind="ExternalInput")
    sinb = nc.dram_tensor("sinb", [128, T], BF16, kind="ExternalInput")
    outT = nc.dram_tensor("outT", [128, 8, T], F32, kind="ExternalOutput")

    with tile.TileContext(nc) as tc:
        with (
            tc.tile_pool(name="sb", bufs=1) as sb,
            tc.tile_pool(name="work", bufs=2) as work,
            tc.tile_pool(name="dsc", bufs=4, space="DRAM") as dsc,
            tc.tile_pool(name="pp", bufs=2, space="PSUM") as pp,
            tc.tile_pool(name="qkp", bufs=2, space="PSUM") as qkp,
            tc.tile_pool(name="avp", bufs=2, space="PSUM") as avp,
        ):
            # ---- resident SBUF tensors ----
            xT_sb = sb.tile([128, CS, T], BF16, name="xT_sb")
            wqk_sb = sb.tile([128, CS, 1024], BF16, name="wqk_sb")
            wv_sb = sb.tile([128, CS, LH * D], BF16, name="wv_sb")
            wpo_sb = sb.tile([128, PAIRS, 1024], BF16, name="wpo_sb")
            bqk_sb = sb.tile([128, 8], F32, name="bqk_sb")
            bv_sb = sb.tile([128, LH * D], F32, name="bv_sb")
            bpo_sb = sb.tile([128, 8], F32, name="bpo_sb")
            cos_sb = sb.tile([128, T], BF16, name="cos_sb")
            sin_sb = sb.tile([128, T], BF16, name="sin_sb")
            vv = sb.tile([128, TT, LH * VW], BF16, name="vv")
            qkr = [sb.tile([128, T], BF16, name=f"qkr{j}") for j in range(8)]
            attnT = [sb.tile([128, T], BF16, name=f"attnT{p}") for p in range(PAIRS)]

            for cs in range(CS):
                nc.sync.dma_start(out=wqk_sb[:, cs, :], in_=wqk[:, cs, :])
                nc.sync.dma_start(out=xT_sb[:, cs, :], in_=xT[:, cs, :])
            nc.sync.dma_start(out=bqk_sb[:], in_=bqk[:])
            nc.sync.dma_start(out=cos_sb[:], in_=cosb[:])
            nc.sync.dma_start(out=sin_sb[:], in_=sinb[:])
            for cs in range(CS):
                nc.sync.dma_start(out=wv_sb[:, cs, :], in_=wv[:, cs, :])
            nc.sync.dma_start(out=bv_sb[:], in_=bvbc[:])
            nc.sync.dma_start(out=wpo_sb[:], in_=wpo[:])
            nc.sync.dma_start(out=bpo_sb[:], in_=bpo[:])

            # ones column of [V|1]
            ones_view = vv.rearrange("p t (h e) -> p t h e", e=VW)[:, :, :, D : D + 1]
            nc.vector.memset(ones_view, 1.0)

            # PE prewarm: ~24 dummy matmuls on a zeroed tile run during the
            # input DMAs, releasing the HAM clock throttle (4/8 -> 8/8)
            warm = sb.tile([128, CH], BF16, name="warm")
            nc.vector.memset(warm[:, :], 0.0)
            wps = pp.tile([128, CH], F32, name="wps", tag="pj")
            for _ in range(10):
                nc.tensor.matmul(
                    wps[:, :], lhsT=warm[:, 0:128], rhs=warm[:, :],
                    start=True, stop=True,
                )

            def qkproj_rope(j):
                """Produce rotated qk_T tile j (j 0-3: Q pairs, 4-7: K pairs).

                Chunk-by-chunk so downstream QK matmuls (which need only one
                roped chunk) unblock as early as possible.  RoPE partner swap
                is a 32-partition block exchange (head dims packed
                [evens | odds] on host)."""
                for c in range(NCHUNK):
                    qkproj_rope_chunk(j, c)

            _qk_stage = {}

            def qkproj_rope_chunk(j, c):
                if j not in _qk_stage:
                    _qk_stage[j] = (
                        work.tile([128, T], BF16, name=f"qp{j}", tag="qp", bufs=3),
                        work.tile([128, T], BF16, name=f"sw{j}", tag="sw", bufs=3),
                    )
                qp, sw = _qk_stage[j]
                if True:
                    cols = slice(c * CH, (c + 1) * CH)
                    pj = pp.tile([128, CH], F32, name="pj", tag="pj")
                    for cs in range(CS):
                        for mh in (0, 1):
                            nc.tensor.matmul(
                                pj[mh * 64 : (mh + 1) * 64, :],
                                lhsT=wqk_sb[
                                    :, cs, j * 128 + mh * 64 : j * 128 + (mh + 1) * 64
                                ],
                                rhs=xT_sb[:, cs, cols],
                                start=(cs == 0),
                                stop=(cs == CS - 1),
                            )
                    nc.vector.tensor_scalar_add(
                        qp[:, cols], pj[:, :], bqk_sb[:, j : j + 1]
                    )
                    for base in (0, 64):
                        nc.sync.dma_start(
                            out=sw[base : base + 32, cols],
                            in_=qp[base + 32 : base + 64, cols],
                        )
                        nc.sync.dma_start(
                            out=sw[base + 32 : base + 64, cols],
                            in_=qp[base : base + 32, cols],
                        )
                    nc.vector.tensor_mul(qp[:, cols], qp[:, cols], cos_sb[:, cols])
                    nc.vector.tensor_mul(sw[:, cols], sw[:, cols], sin_sb[:, cols])
                    nc.vector.tensor_add(qkr[j][:, cols], qp[:, cols], sw[:, cols])

            def vproj_tile(t):
                pj = pp.tile([128, LH * D], F32, name="pj", tag="pj")
                for cs in range(CS):
                    for mh in (0, 1):
                        nc.tensor.matmul(
                            pj[mh * 64 : (mh + 1) * 64, :],
                            lhsT=xT_sb[
                                :, cs, t * 128 + mh * 64 : t * 128 + (mh + 1) * 64
                            ],
                            rhs=wv_sb[:, cs, :],
                            start=(cs == 0),
                            stop=(cs == CS - 1),
                        )
                src = pj.rearrange("p (h e) -> p h e", e=D)
                dst = vv[:, t, :].rearrange("p (h e) -> p h e", e=VW)[:, :, 0:D]
                badd = bv_sb.rearrange("p (h e) -> p h e", e=D)
                nc.vector.tensor_add(dst, src, badd)

            def attention(p, sched=None):
                """sched: {(c, s): [thunk, ...]} -- auxiliary work (projection
                tiles for the next pair, output-projection pieces) emitted at
                chosen s-iterations so the PE absorbs it in ScalarE-bound
                slack instead of lump-stalling the exp pipeline."""
                kt = qkr[4 + p]
                qt = qkr[p]
                for c in range(NCHUNK):
                    pv0 = avp.tile([VW, CH], F32, name="pv0", tag="pv")
                    pv1 = avp.tile([VW, CH], F32, name="pv1", tag="pv")
                    for s in range(TT):
                        if sched:
                            for fn in sched.pop((c, s), ()):
                                fn()
                        sq = qkp.tile([128, 2 * CH], F32, name="sq", tag="sq")
                        # 2x2 PE tiling: heads on row-groups (K=64), s-token
                        # halves on col-groups (M=64).  All four matmuls run
                        # concurrently in disjoint array quadrants and each
                        # LDWEIGHTS is only 64 columns.
                        for h in (0, 1):
                            for mh in (0, 1):
                                nc.tensor.matmul(
                                    sq[mh * 64 : (mh + 1) * 64, h * CH : (h + 1) * CH],
                                    lhsT=kt[
                                        h * 64 : (h + 1) * 64,
                                        s * 128 + mh * 64 : s * 128 + (mh + 1) * 64,
                                    ],
                                    rhs=qt[h * 64 : (h + 1) * 64, c * CH : (c + 1) * CH],
                                    start=True,
                                    stop=True,
                                )
                        ex = work.tile([128, 2 * CH], BF16, name="ex", tag="ex", bufs=3)
                        nc.scalar.activation(
                            out=ex[:, :],
                            in_=sq[:, :],
                            func=mybir.ActivationFunctionType.Exp,
                            scale=SCALE,
                        )
                        for h, pv in ((0, pv0), (1, pv1)):
                            lh = 2 * p + h
                            nc.tensor.matmul(
                                pv[:, :],
                                lhsT=vv[:, s, lh * VW : lh * VW + VW],
                                rhs=ex[:, h * CH : (h + 1) * CH],
                                start=(s == 0),
                                stop=(s == TT - 1),
                            )
                    # normalize: attnT[h rows, chunk] = O'/denom
                    for h, pv in ((0, pv0), (1, pv1)):
                        st = work.tile([VW, CH], F32, name=f"st{h}", tag=f"st{h}")
                        nc.vector.tensor_copy(st[:, :], pv[:, :])
                        # reciprocal is ~8 cyc/free-elem on one lane; bounce the
                        # denominator row through DRAM to fold it onto 64
                        # partitions (FD 512 -> 8) before the exact recip
                        dn = dsc.tile([1, CH], F32, name=f"dn{h}", tag=f"dn{h}")
                        nc.sync.dma_start(out=dn[:, :], in_=st[D : D + 1, :])
                        dv = work.tile([64, CH // 64], F32, name=f"dv{h}", tag=f"dv{h}")
                        dn3 = dn.rearrange("o (p i) -> o p i", p=64)
                        nc.sync.dma_start(out=dv[:, :], in_=dn3[0])
                        nc.vector.reciprocal(dv[:, :], dv[:, :])
                        dn2 = dsc.tile([1, CH], F32, name=f"dm{h}", tag=f"dm{h}")
                        nc.sync.dma_start(
                            out=dn2.rearrange("o (p i) -> o p i", p=64)[0], in_=dv[:, :]
                        )
                        bc = work.tile([64, CH], F32, name=f"bc{h}", tag=f"bc{h}")
                        nc.sync.dma_start(out=bc[:, :], in_=dn2.to_broadcast([64, CH]))
                        if h == 0:
                            nc.vector.tensor_mul(
                                attnT[p][0:64, c * CH : (c + 1) * CH],
                                st[0:64, :],
                                bc[:, :],
                            )
                        else:
                            s1 = work.tile([64, CH], BF16, name="s1", tag="s1")
                            nc.vector.tensor_mul(s1[:, :], st[0:64, :], bc[:, :])
                            nc.sync.dma_start(
                                out=attnT[p][64:128, c * CH : (c + 1) * CH],
                                in_=s1[:, :],
                            )
            def oproj_piece(c, j):
                def emit():
                    pj = pp.tile([128, CH], F32, name="pj", tag="pj")
                    for p in range(PAIRS):
                        for mh in (0, 1):
                            nc.tensor.matmul(
                                pj[mh * 64 : (mh + 1) * 64, :],
                                lhsT=wpo_sb[
                                    :, p, j * 128 + mh * 64 : j * 128 + (mh + 1) * 64
                                ],
                                rhs=attnT[p][:, c * CH : (c + 1) * CH],
                                start=(p == 0),
                                stop=(p == PAIRS - 1),
                            )
                    ob = work.tile([128, CH], F32, name="ob", tag="ob", bufs=3)
                    nc.vector.tensor_scalar_add(
                        ob[:, :], pj[:, :], bpo_sb[:, j : j + 1]
                    )
                    nc.sync.dma_start(
                        out=outT[:, j, c * CH : (c + 1) * CH], in_=ob[:, :]
                    )
                return emit

            # ---- emission plan ----
            # prologue: K pair-0 tile fully (consumed across c0's s loop),
            # Q pair-0 chunk 0 only
            qkproj_rope(4)
            qkproj_rope_chunk(0, 0)

            def prep_piece(j, c):
                return lambda: qkproj_rope_chunk(j, c)

            scheds = [dict() for _ in range(PAIRS)]
            # pair 0 chunk 0: vproj tiles each s + remaining Q0 chunks
            for s in range(TT):
                scheds[0][(0, s)] = [lambda t=s: vproj_tile(t)]
            for cq in (1, 2, 3):
                scheds[0][(0, 4 * cq)].append(prep_piece(0, cq))
            # pairs 0-2: spread the 8 projection chunks of the next pair's
            # Q and K tiles across chunks 1-3 (Q first; K before its use)
            for p in range(PAIRS - 1):
                pieces = [prep_piece(p + 1, c) for c in range(NCHUNK)]
                pieces += [prep_piece(4 + p + 1, c) for c in range(NCHUNK)]
                slots = [(1, 1), (1, 6), (1, 11), (2, 1), (2, 6), (2, 11),
                         (3, 1), (3, 8)]
                for piece, cs_ in zip(pieces, slots):
                    scheds[p].setdefault(cs_, []).append(piece)
            # pair 3: output-projection pieces for chunk c spread across the
            # following chunk's iterations; chunk 3's pieces go to the tail
            tail = []
            for c in range(NCHUNK - 1):
                for j in range(8):
                    scheds[3].setdefault((c + 1, 2 * j), []).append(
                        oproj_piece(c, j)
                    )
            for j in range(8):
                tail.append(oproj_piece(NCHUNK - 1, j))

            for p in range(PAIRS):
                attention(p, sched=scheds[p])
            for fn in tail:
                fn()

    nc.compile()
    return nc


def rope_tables():
    """cos / sign-folded sin tables in [128 partitions, T] layout.

    Head dims are packed [evens | odds]: rows 0-31 hold x0 of pair k=row
    (sinsg = -sin), rows 32-63 hold x1 of pair k=row-32 (sinsg = +sin)."""
    k = np.arange(32).astype(np.float64)
    freqs = ROPE_BASE ** (-2.0 * k / D)  # [32]
    t = np.arange(T, dtype=np.float64)
    theta = t[None, :] * freqs[:, None]  # [32, T]
    cos64 = np.concatenate([np.cos(theta), np.cos(theta)], axis=0)
    sin64 = np.concatenate([-np.sin(theta), np.sin(theta)], axis=0)
    cos128 = np.tile(cos64, (2, 1))
    sin128 = np.tile(sin64, (2, 1))
    return _bf16(cos128), _bf16(sin128)


def pack_group_weights(w_qkv, b_qkv, w_proj, b_proj, g):
    """Per-head-group weight shards in device layout."""
    Wq, Wk, Wv = w_qkv[:, :C], w_qkv[:, C : 2 * C], w_qkv[:, 2 * C :]
    bq, bk, bv = b_qkv[:C], b_qkv[C : 2 * C], b_qkv[2 * C :]
    heads = np.arange(g * LH, (g + 1) * LH)

    # qk tiles: j 0-3 = Q pairs, 4-7 = K pairs; each tile = 2 heads x 64 dims.
    # Within each head the dims are permuted [evens | odds] so the RoPE pair
    # partner is a 32-partition block swap (QK^T invariant to shared perm).
    eo = np.concatenate([np.arange(0, D, 2), np.arange(1, D, 2)])
    qk_cols = []
    bqk_cols = []
    for src, bias in ((Wq, bq), (Wk, bk)):
        for p in range(PAIRS):
            cols = np.concatenate(
                [heads[2 * p] * D + eo, heads[2 * p + 1] * D + eo]
            )
            qk_cols.append(src[:, cols])
            bqk_cols.append(bias[cols])
    wqk_l = np.concatenate(qk_cols, axis=1)  # [C, 1024]
    wqk_dev = _bf16(wqk_l.reshape(CS, 128, 1024).transpose(1, 0, 2))
    bqk_dev = _f32(np.stack(bqk_cols, axis=1))  # [128, 8]

    vcols = np.concatenate([np.arange(h * D, h * D + D) for h in heads])
    wv_dev = _bf16(Wv[:, vcols].reshape(CS, 128, LH * D).transpose(1, 0, 2))
    bv_dev = _f32(np.broadcast_to(bv[vcols], (128, LH * D)))

    # proj rows in attnT order: local index p*128 + e*64 + d <-> head 2p+e
    rows = np.concatenate(
        [np.arange(heads[i] * D, heads[i] * D + D) for i in range(LH)]
    )
    wpo_l = w_proj[rows, :]  # [512, 1024]
    wpo_dev = _bf16(wpo_l.reshape(PAIRS, 128, 1024).transpose(1, 0, 2))

    bpo_full = b_proj if g == 0 else np.zeros_like(b_proj)  # avoid double bias
    bpo_dev = _f32(bpo_full.reshape(8, 128).T)

    return dict(wqk=wqk_dev, bqk=bqk_dev, wv=wv_dev, bvbc=bv_dev,
                wpo=wpo_dev, bpo=bpo_dev)


def make_in_maps(x, w_qkv, b_qkv, w_proj, b_proj):
    x = np.asarray(x, dtype=np.float32)
    w_qkv = np.asarray(w_qkv, dtype=np.float32)
    b_qkv = np.asarray(b_qkv, dtype=np.float32)
    w_proj = np.asarray(w_proj, dtype=np.float32)
    b_proj = np.asarray(b_proj, dtype=np.float32)

    cos_dev, sin_dev = rope_tables()
    gw = [pack_group_weights(w_qkv, b_qkv, w_proj, b_proj, g) for g in (0, 1)]

    in_maps = []
    for core in range(NCORES):
        b, g = core // 2, core % 2
        xT_dev = _bf16(x[b].T.reshape(CS, 128, T).transpose(1, 0, 2))
        m = dict(xT=xT_dev, cosb=cos_dev, sinb=sin_dev, **gw[g])
        in_maps.append(m)
    return in_maps


_NC_CACHE = []


def get_nc():
    if not _NC_CACHE:
        _NC_CACHE.append(build_program())
    return _NC_CACHE[0]


def unshard(results):
    out = np.empty((B, T, C), dtype=np.float32)
    for b in range(B):
        acc = results[2 * b]["outT"].astype(np.float32) + results[2 * b + 1][
            "outT"
        ].astype(np.float32)
        out[b] = acc.transpose(1, 0, 2).reshape(C, T).T
    return out


def run(trace=False, **inputs):
    nc = get_nc()
    in_maps = make_in_maps(**inputs)
    res = run_bass_kernel_spmd(nc, in_maps, core_ids=list(range(NCORES)), trace=trace)
    return unshard(res.results), res


def kernel(**inputs) -> np.ndarray:
    out, _ = run(trace=False, **inputs)
    return out



# revision 8
# speedup vs baseline: 1.3939x; 1.3939x over previous
# Trainium2 Bass kernel for nn_MHA_18657337934739
#
# MHA: qkv = x@Wqkv + b; q,k = rope(q),rope(k); softmax(q k^T / 8) @ v; proj.
# Shapes: B=4, T=2048, C=1024, H=16 heads, D=64.
#
# Sharding: 8 cores = (4 batches) x (2 head-groups of 8 heads).  Each core
# computes its batch's attention for its 8 heads plus the partial output
# projection (contraction over its 512 local channels).  Host sums the two
# partials per batch (tensor-parallel unshard) and transposes back.
#
# On-core dataflow (all matmul inputs bf16, PSUM accumulation f32):
#   qk_T[c', t] = Wqk_loc^T x^T   (channels on partitions -> RoPE via
#                                  partition-pair swap DMA + cos/sin tables)
#   v[t, d]     = x Wv_loc        (tokens on partitions; +ones column)
#   S_T[s, q]   = K_rot^T Q_rot   (row-tiled pairs: two K=64 matmuls share
#                                  the PE array via tile_position rows 0/64)
#   P = exp(S_T / 8)              (ScalarE, no max-subtraction: |S|<~4)
#   O'[d+1, q]  = [V|1]^T P       (M=65: row 64 = softmax denominator)
#   attnT       = O'[0:64]/denom  (recip + partition-broadcast via DRAM)
#   out_T       = Wproj_loc^T attnT + b  (partial; host sums group pairs)

import numpy as np
import ml_dtypes

import concourse.bass as bass
import concourse.tile as tile
from concourse import bacc, mybir
from concourse.bass_utils import run_bass_kernel_spmd

BF16 = mybir.dt.bfloat16
F32 = mybir.dt.float32

B, T, C = 4, 2048, 1024
H, D = 16, 64
ROPE_BASE = 10000.0
SCALE = 1.0 / 8.0  # 1/sqrt(D)

NCORES = 8
LH = 8          # local heads per core
PAIRS = LH // 2  # 4
CS = C // 128    # 8 contraction subtiles
TT = T // 128    # 16 token tiles
CH = 512         # q-chunk width
NCHUNK = T // CH  # 4
VW = D + 1       # 65: V plus ones column


def _bf16(a):
    return np.ascontiguousarray(a).astype(ml_dtypes.bfloat16)


def _f32(a):
    return np.ascontiguousarray(a).astype(np.float32)


def build_program():
    nc = bacc.Bacc("TRN2", target_bir_lowering=False, debug=False)

    xT = nc.dram_tensor("xT", [128, CS, T], BF16, kind="ExternalInput")
    wqk = nc.dram_tensor("wqk", [128, CS, 1024], BF16, kind="ExternalInput")
    wv = nc.dram_tensor("wv", [128, CS, LH * D], BF16, kind="ExternalInput")
    wpo = nc.dram_tensor("wpo", [128, PAIRS, 1024], BF16, kind="ExternalInput")
    bqk = nc.dram_tensor("bqk", [128, 8], F32, kind="ExternalInput")
    bvbc = nc.dram_tensor("bvbc", [128, LH * D], F32, kind="ExternalInput")
    bpo = nc.dram_tensor("bpo", [128, 8], F32, kind="ExternalInput")
    cosb = nc.dram_tensor("cosb", [128, T], BF16, kind="ExternalInput")
    sinb = nc.dram_tensor("sinb", [128, T], BF16, kind="ExternalInput")
    outT = nc.dram_tensor("outT", [128, 8, T], F32, kind="ExternalOutput")

    with tile.TileContext(nc) as tc:
        with (
            tc.tile_pool(name="sb", bufs=1) as sb,
            tc.tile_pool(name="work", bufs=2) as work,
            tc.tile_pool(name="dsc", bufs=4, space="DRAM") as dsc,
            tc.tile_pool(name="pp", bufs=2, space="PSUM") as pp,
            tc.tile_pool(name="qkp", bufs=2, space="PSUM") as qkp,
            tc.tile_pool(name="avp", bufs=2, space="PSUM") as avp,
        ):
            # ---- resident SBUF tensors ----
            xT_sb = sb.tile([128, CS, T], BF16, name="xT_sb")
            wqk_sb = sb.tile([128, CS, 1024], BF16, name="wqk_sb")
            wv_sb = sb.tile([128, CS, LH * D], BF16, name="wv_sb")
            wpo_sb = sb.tile([128, PAIRS, 1024], BF16, name="wpo_sb")
            bqk_sb = sb.tile([128, 8], F32, name="bqk_sb")
            bv_sb = sb.tile([128, LH * D], F32, name="bv_sb")
            bpo_sb = sb.tile([128, 8], F32, name="bpo_sb")
            cos_sb = sb.tile([128, T], BF16, name="cos_sb")
            sin_sb = sb.tile([128, T], BF16, name="sin_sb")
            vv = sb.tile([128, TT, LH * VW], BF16, name="vv")
            qkr = [sb.tile([128, T], BF16, name=f"qkr{j}") for j in range(8)]
            attnT = [sb.tile([128, T], BF16, name=f"attnT{p}") for p in range(PAIRS)]

            for cs in range(CS):
                nc.sync.dma_start(out=wqk_sb[:, cs, :], in_=wqk[:, cs, :])
                nc.sync.dma_start(out=xT_sb[:, cs, :], in_=xT[:, cs, :])
            nc.sync.dma_start(out=bqk_sb[:], in_=bqk[:])
            nc.sync.dma_start(out=cos_sb[:], in_=cosb[:])
            nc.sync.dma_start(out=sin_sb[:], in_=sinb[:])
            for cs in range(CS):
                nc.sync.dma_start(out=wv_sb[:, cs, :], in_=wv[:, cs, :])
            nc.sync.dma_start(out=bv_sb[:], in_=bvbc[:])
            nc.sync.dma_start(out=wpo_sb[:], in_=wpo[:])
            nc.sync.dma_start(out=bpo_sb[:], in_=bpo[:])

            # ones column of [V|1]
            ones_view = vv.rearrange("p t (h e) -> p t h e", e=VW)[:, :, :, D : D + 1]
            nc.vector.memset(ones_view, 1.0)

            # PE prewarm: ~24 dummy matmuls on a zeroed tile run during the
            # input DMAs, releasing the HAM clock throttle (4/8 -> 8/8)
            warm = sb.tile([128, CH], BF16, name="warm")
            nc.vector.memset(warm[:, :], 0.0)
            wps = pp.tile([128, CH], F32, name="wps", tag="pj")
            for _ in range(10):
                nc.tensor.matmul(
                    wps[:, :], lhsT=warm[:, 0:128], rhs=warm[:, :],
                    start=True, stop=True,
                )

            def qkproj_rope(j):
                """Produce rotated qk_T tile j (j 0-3: Q pairs, 4-7: K pairs).

                Chunk-by-chunk so downstream QK matmuls (which need only one
                roped chunk) unblock as early as possible.  RoPE partner swap
                is a 32-partition block exchange (head dims packed
                [evens | odds] on host)."""
                for c in range(NCHUNK):
                    qkproj_rope_chunk(j, c)

            _qk_stage = {}

            def qkproj_rope_chunk(j, c):
                if j not in _qk_stage:
                    _qk_stage[j] = (
                        work.tile([128, T], BF16, name=f"qp{j}", tag="qp", bufs=3),
                        work.tile([128, T], BF16, name=f"sw{j}", tag="sw", bufs=3),
                    )
                qp, sw = _qk_stage[j]
                if True:
                    cols = slice(c * CH, (c + 1) * CH)
                    pj = pp.tile([128, CH], F32, name="pj", tag="pj")
                    for cs in range(CS):
                        nc.tensor.matmul(
                            pj[:, :],
                            lhsT=wqk_sb[:, cs, j * 128 : (j + 1) * 128],
                            rhs=xT_sb[:, cs, cols],
                            start=(cs == 0),
                            stop=(cs == CS - 1),
                        )
                    nc.vector.tensor_scalar_add(
                        qp[:, cols], pj[:, :], bqk_sb[:, j : j + 1]
                    )
                    for base in (0, 64):
                        nc.sync.dma_start(
                            out=sw[base : base + 32, cols],
                            in_=qp[base + 32 : base + 64, cols],
                        )
                        nc.sync.dma_start(
                            out=sw[base + 32 : base + 64, cols],
                            in_=qp[base : base + 32, cols],
                        )
                    nc.vector.tensor_mul(qp[:, cols], qp[:, cols], cos_sb[:, cols])
                    nc.vector.tensor_mul(sw[:, cols], sw[:, cols], sin_sb[:, cols])
                    nc.vector.tensor_add(qkr[j][:, cols], qp[:, cols], sw[:, cols])

            def vproj_tile(t):
                pj = pp.tile([128, LH * D], F32, name="pj", tag="pj")
                for cs in range(CS):
                    nc.tensor.matmul(
                        pj[:, :],
                        lhsT=xT_sb[:, cs, t * 128 : (t + 1) * 128],
                        rhs=wv_sb[:, cs, :],
                        start=(cs == 0),
                        stop=(cs == CS - 1),
                    )
                src = pj.rearrange("p (h e) -> p h e", e=D)
                dst = vv[:, t, :].rearrange("p (h e) -> p h e", e=VW)[:, :, 0:D]
                badd = bv_sb.rearrange("p (h e) -> p h e", e=D)
                nc.vector.tensor_add(dst, src, badd)

            def attention(p, sched=None):
                """sched: {(c, s): [thunk, ...]} -- auxiliary work (projection
                tiles for the next pair, output-projection pieces) emitted at
                chosen s-iterations so the PE absorbs it in ScalarE-bound
                slack instead of lump-stalling the exp pipeline."""
                kt = qkr[4 + p]
                qt = qkr[p]
                for c in range(NCHUNK):
                    pv0 = avp.tile([VW, CH], F32, name="pv0", tag="pv")
                    pv1 = avp.tile([VW, CH], F32, name="pv1", tag="pv")

                    def av_mms(s, ex):
                        for h, pv in ((0, pv0), (1, pv1)):
                            lh = 2 * p + h
                            nc.tensor.matmul(
                                pv[:, :],
                                lhsT=vv[:, s, lh * VW : lh * VW + VW],
                                rhs=ex[:, h * CH : (h + 1) * CH],
                                start=(s == 0),
                                stop=(s == TT - 1),
                            )

                    # AV for iteration s is emitted AFTER exp(s+1) so the PE
                    # instruction order is [... QK(s+1), AV(s) ...]: exp(s+1)'s
                    # PE-tick wait covers only QK(s+1), and AV(s) streams
                    # during exp(s+1) off the previous ex buffer.
                    pend = None
                    for s in range(TT):
                        if sched:
                            for fn in sched.pop((c, s), ()):
                                fn()
                        sq = qkp.tile([128, 2 * CH], F32, name="sq", tag="sq")
                        for h in (0, 1):
                            nc.tensor.matmul(
                                sq[:, h * CH : (h + 1) * CH],
                                lhsT=kt[h * 64 : (h + 1) * 64, s * 128 : (s + 1) * 128],
                                rhs=qt[h * 64 : (h + 1) * 64, c * CH : (c + 1) * CH],
                                start=True,
                                stop=True,
                            )
                        ex = work.tile([128, 2 * CH], BF16, name="ex", tag="ex", bufs=3)
                        nc.scalar.activation(
                            out=ex[:, :],
                            in_=sq[:, :],
                            func=mybir.ActivationFunctionType.Exp,
                            scale=SCALE,
                        )
                        if pend is not None:
                            av_mms(*pend)
                        pend = (s, ex)
                    av_mms(*pend)
                    # normalize: attnT[h rows, chunk] = O'/denom
                    for h, pv in ((0, pv0), (1, pv1)):
                        st = work.tile([VW, CH], F32, name=f"st{h}", tag=f"st{h}")
                        nc.vector.tensor_copy(st[:, :], pv[:, :])
                        # reciprocal is ~8 cyc/free-elem on one lane; bounce the
                        # denominator row through DRAM to fold it onto 64
                        # partitions (FD 512 -> 8) before the exact recip
                        dn = dsc.tile([1, CH], F32, name=f"dn{h}", tag=f"dn{h}")
                        nc.sync.dma_start(out=dn[:, :], in_=st[D : D + 1, :])
                        dv = work.tile([64, CH // 64], F32, name=f"dv{h}", tag=f"dv{h}")
                        dn3 = dn.rearrange("o (p i) -> o p i", p=64)
                        nc.sync.dma_start(out=dv[:, :], in_=dn3[0])
                        nc.vector.reciprocal(dv[:, :], dv[:, :])
                        dn2 = dsc.tile([1, CH], F32, name=f"dm{h}", tag=f"dm{h}")
                        nc.sync.dma_start(
                            out=dn2.rearrange("o (p i) -> o p i", p=64)[0], in_=dv[:, :]
                        )
                        bc = work.tile([64, CH], F32, name=f"bc{h}", tag=f"bc{h}")
                        nc.sync.dma_start(out=bc[:, :], in_=dn2.to_broadcast([64, CH]))
                        if h == 0:
                            nc.vector.tensor_mul(
                                attnT[p][0:64, c * CH : (c + 1) * CH],
                                st[0:64, :],
                                bc[:, :],
                            )
                        else:
                            s1 = work.tile([64, CH], BF16, name="s1", tag="s1")
                            nc.vector.tensor_mul(s1[:, :], st[0:64, :], bc[:, :])
                            nc.sync.dma_start(
                                out=attnT[p][64:128, c * CH : (c + 1) * CH],
                                in_=s1[:, :],
                            )
            def oproj_piece(c, j):
                def emit():
                    pj = pp.tile([128, CH], F32, name="pj", tag="pj")
                    for p in range(PAIRS):
                        nc.tensor.matmul(
                            pj[:, :],
                            lhsT=wpo_sb[:, p, j * 128 : (j + 1) * 128],
                            rhs=attnT[p][:, c * CH : (c + 1) * CH],
                            start=(p == 0),
                            stop=(p == PAIRS - 1),
                        )
                    ob = work.tile([128, CH], F32, name="ob", tag="ob", bufs=3)
                    nc.vector.tensor_scalar_add(
                        ob[:, :], pj[:, :], bpo_sb[:, j : j + 1]
                    )
                    nc.sync.dma_start(
                        out=outT[:, j, c * CH : (c + 1) * CH], in_=ob[:, :]
                    )
                return emit

            # ---- emission plan ----
            # prologue: K pair-0 tile fully (consumed across c0's s loop),
            # Q pair-0 chunk 0 only
            qkproj_rope(4)
            qkproj_rope_chunk(0, 0)

            def prep_piece(j, c):
                return lambda: qkproj_rope_chunk(j, c)

            scheds = [dict() for _ in range(PAIRS)]
            # pair 0 chunk 0: vproj tiles each s + remaining Q0 chunks
            for s in range(TT):
                scheds[0][(0, s)] = [lambda t=s: vproj_tile(t)]
            for cq in (1, 2, 3):
                scheds[0][(0, 4 * cq)].append(prep_piece(0, cq))
            # pairs 0-2: spread the 8 projection chunks of the next pair's
            # Q and K tiles across chunks 1-3 (Q first; K before its use)
            for p in range(PAIRS - 1):
                pieces = [prep_piece(p + 1, c) for c in range(NCHUNK)]
                pieces += [prep_piece(4 + p + 1, c) for c in range(NCHUNK)]
                slots = [(1, 1), (1, 6), (1, 11), (2, 1), (2, 6), (2, 11),
                         (3, 1), (3, 8)]
                for piece, cs_ in zip(pieces, slots):
                    scheds[p].setdefault(cs_, []).append(piece)
            # pair 3: output-projection pieces for chunk c spread across the
            # following chunk's iterations; chunk 3's pieces go to the tail
            tail = []
            for c in range(NCHUNK - 1):
                for j in range(8):
                    scheds[3].setdefault((c + 1, 2 * j), []).append(
                        oproj_piece(c, j)
                    )
            for j in range(8):
                tail.append(oproj_piece(NCHUNK - 1, j))

            for p in range(PAIRS):
                attention(p, sched=scheds[p])
            for fn in tail:
                fn()

    nc.compile()
    return nc


def rope_tables():
    """cos / sign-folded sin tables in [128 partitions, T] layout.

    Head dims are packed [evens | odds]: rows 0-31 hold x0 of pair k=row
    (sinsg = -sin), rows 32-63 hold x1 of pair k=row-32 (sinsg = +sin)."""
    k = np.arange(32).astype(np.float64)
    freqs = ROPE_BASE ** (-2.0 * k / D)  # [32]
    t = np.arange(T, dtype=np.float64)
    theta = t[None, :] * freqs[:, None]  # [32, T]
    cos64 = np.concatenate([np.cos(theta), np.cos(theta)], axis=0)
    sin64 = np.concatenate([-np.sin(theta), np.sin(theta)], axis=0)
    cos128 = np.tile(cos64, (2, 1))
    sin128 = np.tile(sin64, (2, 1))
    return _bf16(cos128), _bf16(sin128)


def pack_group_weights(w_qkv, b_qkv, w_proj, b_proj, g):
    """Per-head-group weight shards in device layout."""
    Wq, Wk, Wv = w_qkv[:, :C], w_qkv[:, C : 2 * C], w_qkv[:, 2 * C :]
    bq, bk, bv = b_qkv[:C], b_qkv[C : 2 * C], b_qkv[2 * C :]
    heads = np.arange(g * LH, (g + 1) * LH)

    # qk tiles: j 0-3 = Q pairs, 4-7 = K pairs; each tile = 2 heads x 64 dims.
    # Within each head the dims are permuted [evens | odds] so the RoPE pair
    # partner is a 32-partition block swap (QK^T invariant to shared perm).
    eo = np.concatenate([np.arange(0, D, 2), np.arange(1, D, 2)])
    qk_cols = []
    bqk_cols = []
    for src, bias in ((Wq, bq), (Wk, bk)):
        for p in range(PAIRS):
            cols = np.concatenate(
                [heads[2 * p] * D + eo, heads[2 * p + 1] * D + eo]
            )
            qk_cols.append(src[:, cols])
            bqk_cols.append(bias[cols])
    wqk_l = np.concatenate(qk_cols, axis=1)  # [C, 1024]
    wqk_dev = _bf16(wqk_l.reshape(CS, 128, 1024).transpose(1, 0, 2))
    bqk_dev = _f32(np.stack(bqk_cols, axis=1))  # [128, 8]

    vcols = np.concatenate([np.arange(h * D, h * D + D) for h in heads])
    wv_dev = _bf16(Wv[:, vcols].reshape(CS, 128, LH * D).transpose(1, 0, 2))
    bv_dev = _f32(np.broadcast_to(bv[vcols], (128, LH * D)))

    # proj rows in attnT order: local index p*128 + e*64 + d <-> head 2p+e
    rows = np.concatenate(
        [np.arange(heads[i] * D, heads[i] * D + D) for i in range(LH)]
    )
    wpo_l = w_proj[rows, :]  # [512, 1024]
    wpo_dev = _bf16(wpo_l.reshape(PAIRS, 128, 1024).transpose(1, 0, 2))

    bpo_full = b_proj if g == 0 else np.zeros_like(b_proj)  # avoid double bias
    bpo_dev = _f32(bpo_full.reshape(8, 128).T)

    return dict(wqk=wqk_dev, bqk=bqk_dev, wv=wv_dev, bvbc=bv_dev,
                wpo=wpo_dev, bpo=bpo_dev)


def make_in_maps(x, w_qkv, b_qkv, w_proj, b_proj):
    x = np.asarray(x, dtype=np.float32)
    w_qkv = np.asarray(w_qkv, dtype=np.float32)
    b_qkv = np.asarray(b_qkv, dtype=np.float32)
    w_proj = np.asarray(w_proj, dtype=np.float32)
    b_proj = np.asarray(b_proj, dtype=np.float32)

    cos_dev, sin_dev = rope_tables()
    gw = [pack_group_weights(w_qkv, b_qkv, w_proj, b_proj, g) for g in (0, 1)]

    in_maps = []
    for core in range(NCORES):
        b, g = core // 2, core % 2
        xT_dev = _bf16(x[b].T.reshape(CS, 128, T).transpose(1, 0, 2))
        m = dict(xT=xT_dev, cosb=cos_dev, sinb=sin_dev, **gw[g])
        in_maps.append(m)
    return in_maps


_NC_CACHE = []


def get_nc():
    if not _NC_CACHE:
        _NC_CACHE.append(build_program())
    return _NC_CACHE[0]


def unshard(results):
    out = np.empty((B, T, C), dtype=np.float32)
    for b in range(B):
        acc = results[2 * b]["outT"].astype(np.float32) + results[2 * b + 1][
            "outT"
        ].astype(np.float32)
        out[b] = acc.transpose(1, 0, 2).reshape(C, T).T
    return out


def run(trace=False, **inputs):
    nc = get_nc()
    in_maps = make_in_maps(**inputs)
    res = run_bass_kernel_spmd(nc, in_maps, core_ids=list(range(NCORES)), trace=trace)
    return unshard(res.results), res


def kernel(**inputs) -> np.ndarray:
    out, _ = run(trace=False, **inputs)
    return out



# revision 11
# speedup vs baseline: 1.4106x; 1.0119x over previous
# Trainium2 Bass kernel for nn_MHA_18657337934739
#
# MHA: qkv = x@Wqkv + b; q,k = rope(q),rope(k); softmax(q k^T / 8) @ v; proj.
# Shapes: B=4, T=2048, C=1024, H=16 heads, D=64.
#
# Sharding: 8 cores = (4 batches) x (2 head-groups of 8 heads).  Each core
# computes its batch's attention for its 8 heads plus the partial output
# projection (contraction over its 512 local channels).  Host sums the two
# partials per batch (tensor-parallel unshard) and transposes back.
#
# On-core dataflow (all matmul inputs bf16, PSUM accumulation f32):
#   qk_T[c', t] = Wqk_loc^T x^T   (channels on partitions -> RoPE via
#                                  partition-pair swap DMA + cos/sin tables)
#   v[t, d]     = x Wv_loc        (tokens on partitions; +ones column)
#   S_T[s, q]   = K_rot^T Q_rot   (row-tiled pairs: two K=64 matmuls share
#                                  the PE array via tile_position rows 0/64)
#   P = exp(S_T / 8)              (ScalarE, no max-subtraction: |S|<~4)
#   O'[d+1, q]  = [V|1]^T P       (M=65: row 64 = softmax denominator)
#   attnT       = O'[0:64]/denom  (recip + partition-broadcast via DRAM)
#   out_T       = Wproj_loc^T attnT + b  (partial; host sums group pairs)

import numpy as np
import ml_dtypes

import concourse.bass as bass
import concourse.tile as tile
from concourse import bacc, mybir
from concourse.bass_utils import run_bass_kernel_spmd

BF16 = mybir.dt.bfloat16
F32 = mybir.dt.float32

B, T, C = 4, 2048, 1024
H, D = 16, 64
ROPE_BASE = 10000.0
SCALE = 1.0 / 8.0  # 1/sqrt(D)

NCORES = 8
LH = 8          # local heads per core
PAIRS = LH // 2  # 4
CS = C // 128    # 8 contraction subtiles
TT = T // 128    # 16 token tiles
CH = 512         # q-chunk width
NCHUNK = T // CH  # 4
VW = D + 1       # 65: V plus ones column


def _bf16(a):
    return np.ascontiguousarray(a).astype(ml_dtypes.bfloat16)


def _f32(a):
    return np.ascontiguousarray(a).astype(np.float32)


def build_program():
    nc = bacc.Bacc("TRN2", target_bir_lowering=False, debug=False)

    xT = nc.dram_tensor("xT", [128, CS, T], BF16, kind="ExternalInput")
    wqk = nc.dram_tensor("wqk", [128, 8, CS, 128], BF16, kind="ExternalInput")
    wv = nc.dram_tensor("wv", [128, CS, LH * D], BF16, kind="ExternalInput")
    wpo = nc.dram_tensor("wpo", [128, PAIRS, 1024], BF16, kind="ExternalInput")
    bqk = nc.dram_tensor("bqk", [128, 8], F32, kind="ExternalInput")
    bvbc = nc.dram_tensor("bvbc", [128, LH * D], F32, kind="ExternalInput")
    bpo = nc.dram_tensor("bpo", [128, 8], F32, kind="ExternalInput")
    cosb = nc.dram_tensor("cosb", [128, T], BF16, kind="ExternalInput")
    sinb = nc.dram_tensor("sinb", [128, T], BF16, kind="ExternalInput")
    outT = nc.dram_tensor("outT", [128, 8, T], F32, kind="ExternalOutput")

    with tile.TileContext(nc) as tc:
        with (
            tc.tile_pool(name="sb", bufs=1) as sb,
            tc.tile_pool(name="work", bufs=2) as work,
            tc.tile_pool(name="dsc", bufs=4, space="DRAM") as dsc,
            tc.tile_pool(name="pp", bufs=2, space="PSUM") as pp,
            tc.tile_pool(name="qkp", bufs=2, space="PSUM") as qkp,
            tc.tile_pool(name="avp", bufs=2, space="PSUM") as avp,
        ):
            # ---- resident SBUF tensors ----
            xT_sb = sb.tile([128, CS, T], BF16, name="xT_sb")
            wqk_sb = sb.tile([128, 8, CS, 128], BF16, name="wqk_sb")
            wv_sb = sb.tile([128, CS, LH * D], BF16, name="wv_sb")
            wpo_sb = sb.tile([128, PAIRS, 1024], BF16, name="wpo_sb")
            bqk_sb = sb.tile([128, 8], F32, name="bqk_sb")
            bv_sb = sb.tile([128, LH * D], F32, name="bv_sb")
            bpo_sb = sb.tile([128, 8], F32, name="bpo_sb")
            cos_sb = sb.tile([128, T], BF16, name="cos_sb")
            sin_sb = sb.tile([128, T], BF16, name="sin_sb")
            vv = sb.tile([128, TT, LH * VW], BF16, name="vv")
            qkr = [sb.tile([128, T], BF16, name=f"qkr{j}") for j in range(8)]
            attnT = [sb.tile([128, T], BF16, name=f"attnT{p}") for p in range(PAIRS)]

            # DMA issue order tracks first use: K-pair-0 weights, then x,
            # then Q-pair-0 weights + rope tables, then V, then the rest.
            nc.sync.dma_start(out=wqk_sb[:, 4], in_=wqk[:, 4])
            for cs in range(CS):
                nc.sync.dma_start(out=xT_sb[:, cs, :], in_=xT[:, cs, :])
            nc.sync.dma_start(out=wqk_sb[:, 0], in_=wqk[:, 0])
            nc.sync.dma_start(out=cos_sb[:], in_=cosb[:])
            nc.sync.dma_start(out=sin_sb[:], in_=sinb[:])
            nc.sync.dma_start(out=bqk_sb[:], in_=bqk[:])
            for cs in range(CS):
                nc.sync.dma_start(out=wv_sb[:, cs, :], in_=wv[:, cs, :])
            nc.sync.dma_start(out=bv_sb[:], in_=bvbc[:])
            for j in (5, 1, 6, 2, 7, 3):
                nc.sync.dma_start(out=wqk_sb[:, j], in_=wqk[:, j])
            nc.sync.dma_start(out=wpo_sb[:], in_=wpo[:])
            nc.sync.dma_start(out=bpo_sb[:], in_=bpo[:])

            # ones column of [V|1]
            ones_view = vv.rearrange("p t (h e) -> p t h e", e=VW)[:, :, :, D : D + 1]
            nc.vector.memset(ones_view, 1.0)

            # PE prewarm: ~24 dummy matmuls on a zeroed tile run during the
            # input DMAs, releasing the HAM clock throttle (4/8 -> 8/8)
            warm = sb.tile([128, CH], BF16, name="warm")
            nc.vector.memset(warm[:, :], 0.0)
            wps = pp.tile([128, CH], F32, name="wps", tag="pj")
            for _ in range(10):
                nc.tensor.matmul(
                    wps[:, :], lhsT=warm[:, 0:128], rhs=warm[:, :],
                    start=True, stop=True,
                )

            # ---- fragmented aux pieces ----
            # Each projection piece (8 or 4 accumulating matmuls + a finish)
            # is split into small units so the attention loop can absorb them
            # in per-iteration ScalarE slack without stalling the exp chain.
            # The shared "pj" PSUM slot pair (bufs=2) requires pieces to be
            # emitted strictly sequentially (a finish before the 2nd-next
            # piece opens).
            open_pj = {}

            def pj_tile(key):
                if key not in open_pj:
                    assert len(open_pj) < 2, f"pj over-subscribed: {list(open_pj)}"
                    open_pj[key] = pp.tile([128, CH], F32, name="pj", tag="pj")
                return open_pj[key]

            _qk_stage = {}

            def qkproj_mms(j, c, cs_list):
                pj = pj_tile(("qk", j, c))
                cols = slice(c * CH, (c + 1) * CH)
                for cs in cs_list:
                    nc.tensor.matmul(
                        pj[:, :],
                        lhsT=wqk_sb[:, j, cs, :],
                        rhs=xT_sb[:, cs, cols],
                        start=(cs == 0),
                        stop=(cs == CS - 1),
                    )

            def qkproj_finish(j, c):
                """bias add + RoPE (partner swap via 32-partition block DMA,
                cos/sin tables with sign folded into sin)."""
                pj = open_pj.pop(("qk", j, c))
                if j not in _qk_stage:
                    _qk_stage[j] = (
                        work.tile([128, T], BF16, name=f"qp{j}", tag="qp", bufs=3),
                        work.tile([128, T], BF16, name=f"sw{j}", tag="sw", bufs=3),
                    )
                qp, sw = _qk_stage[j]
                cols = slice(c * CH, (c + 1) * CH)
                nc.vector.tensor_scalar_add(
                    qp[:, cols], pj[:, :], bqk_sb[:, j : j + 1]
                )
                for base in (0, 64):
                    nc.sync.dma_start(
                        out=sw[base : base + 32, cols],
                        in_=qp[base + 32 : base + 64, cols],
                    )
                    nc.sync.dma_start(
                        out=sw[base + 32 : base + 64, cols],
                        in_=qp[base : base + 32, cols],
                    )
                nc.vector.tensor_mul(qp[:, cols], qp[:, cols], cos_sb[:, cols])
                nc.vector.tensor_mul(sw[:, cols], sw[:, cols], sin_sb[:, cols])
                nc.vector.tensor_add(qkr[j][:, cols], qp[:, cols], sw[:, cols])

            def vproj_mms(t, cs_list):
                pj = pj_tile(("v", t))
                for cs in cs_list:
                    nc.tensor.matmul(
                        pj[:, 0 : LH * D],
                        lhsT=xT_sb[:, cs, t * 128 : (t + 1) * 128],
                        rhs=wv_sb[:, cs, :],
                        start=(cs == 0),
                        stop=(cs == CS - 1),
                    )

            def vproj_finish(t):
                pj = open_pj.pop(("v", t))
                src = pj[:, 0 : LH * D].rearrange("p (h e) -> p h e", e=D)
                dst = vv[:, t, :].rearrange("p (h e) -> p h e", e=VW)[:, :, 0:D]
                badd = bv_sb.rearrange("p (h e) -> p h e", e=D)
                nc.vector.tensor_add(dst, src, badd)

            CS_UNITS = [(0, 1), (2, 3), (4, 5), (6, 7)]

            def qk_units(j, c):
                units = [
                    lambda cl=cl: qkproj_mms(j, c, cl) for cl in CS_UNITS
                ]
                last = units[-1]
                units[-1] = lambda: (last(), qkproj_finish(j, c))
                return units

            def vp_units(t):
                units = [lambda cl=cl: vproj_mms(t, cl) for cl in CS_UNITS]
                last = units[-1]
                units[-1] = lambda: (last(), vproj_finish(t))
                return units

            def run_units(units):
                for u in units:
                    u()

            def attention(p, sched=None):
                """sched: {(c, s): [thunk, ...]} -- auxiliary work (projection
                tiles for the next pair, output-projection pieces) emitted at
                chosen s-iterations so the PE absorbs it in ScalarE-bound
                slack instead of lump-stalling the exp pipeline."""
                kt = qkr[4 + p]
                qt = qkr[p]
                for c in range(NCHUNK):
                    pv0 = avp.tile([VW, CH], F32, name="pv0", tag="pv")
                    pv1 = avp.tile([VW, CH], F32, name="pv1", tag="pv")

                    def av_mms(s, ex):
                        for h, pv in ((0, pv0), (1, pv1)):
                            lh = 2 * p + h
                            nc.tensor.matmul(
                                pv[:, :],
                                lhsT=vv[:, s, lh * VW : lh * VW + VW],
                                rhs=ex[:, h * CH : (h + 1) * CH],
                                start=(s == 0),
                                stop=(s == TT - 1),
                            )

                    # AV for iteration s is emitted AFTER exp(s+1) so the PE
                    # instruction order is [... QK(s+1), AV(s) ...]: exp(s+1)'s
                    # PE-tick wait covers only QK(s+1), and AV(s) streams
                    # during exp(s+1) off the previous ex buffer.
                    pend = None
                    for s in range(TT):
                        units = sched.pop((c, s), ()) if sched else ()
                        if units:
                            units[0]()
                        sq = qkp.tile([128, 2 * CH], F32, name="sq", tag="sq")
                        for h in (0, 1):
                            nc.tensor.matmul(
                                sq[:, h * CH : (h + 1) * CH],
                                lhsT=kt[h * 64 : (h + 1) * 64, s * 128 : (s + 1) * 128],
                                rhs=qt[h * 64 : (h + 1) * 64, c * CH : (c + 1) * CH],
                                start=True,
                                stop=True,
                            )
                        ex = work.tile([128, 2 * CH], BF16, name="ex", tag="ex", bufs=3)
                        nc.scalar.activation(
                            out=ex[:, :],
                            in_=sq[:, :],
                            func=mybir.ActivationFunctionType.Exp,
                            scale=SCALE,
                        )
                        if pend is not None:
                            av_mms(*pend)
                        pend = (s, ex)
                        for fn in units[1:]:
                            fn()
                    av_mms(*pend)
                    # normalize: attnT[h rows, chunk] = O'/denom
                    for h, pv in ((0, pv0), (1, pv1)):
                        st = work.tile([VW, CH], F32, name=f"st{h}", tag=f"st{h}")
                        nc.vector.tensor_copy(st[:, :], pv[:, :])
                        # reciprocal is ~8 cyc/free-elem on one lane; bounce the
                        # denominator row through DRAM to fold it onto 64
                        # partitions (FD 512 -> 8) before the exact recip
                        dn = dsc.tile([1, CH], F32, name=f"dn{h}", tag=f"dn{h}")
                        nc.sync.dma_start(out=dn[:, :], in_=st[D : D + 1, :])
                        dv = work.tile([64, CH // 64], F32, name=f"dv{h}", tag=f"dv{h}")
                        dn3 = dn.rearrange("o (p i) -> o p i", p=64)
                        nc.sync.dma_start(out=dv[:, :], in_=dn3[0])
                        nc.vector.reciprocal(dv[:, :], dv[:, :])
                        dn2 = dsc.tile([1, CH], F32, name=f"dm{h}", tag=f"dm{h}")
                        nc.sync.dma_start(
                            out=dn2.rearrange("o (p i) -> o p i", p=64)[0], in_=dv[:, :]
                        )
                        bc = work.tile([64, CH], F32, name=f"bc{h}", tag=f"bc{h}")
                        nc.sync.dma_start(out=bc[:, :], in_=dn2.to_broadcast([64, CH]))
                        if h == 0:
                            nc.vector.tensor_mul(
                                attnT[p][0:64, c * CH : (c + 1) * CH],
                                st[0:64, :],
                                bc[:, :],
                            )
                        else:
                            s1 = work.tile([64, CH], BF16, name="s1", tag="s1")
                            nc.vector.tensor_mul(s1[:, :], st[0:64, :], bc[:, :])
                            nc.sync.dma_start(
                                out=attnT[p][64:128, c * CH : (c + 1) * CH],
                                in_=s1[:, :],
                            )
            def oproj_mms(c, j, ps):
                pj = pj_tile(("o", c, j))
                for p in ps:
                    nc.tensor.matmul(
                        pj[:, :],
                        lhsT=wpo_sb[:, p, j * 128 : (j + 1) * 128],
                        rhs=attnT[p][:, c * CH : (c + 1) * CH],
                        start=(p == 0),
                        stop=(p == PAIRS - 1),
                    )

            def oproj_finish(c, j):
                pj = open_pj.pop(("o", c, j))
                ob = work.tile([128, CH], F32, name="ob", tag="ob", bufs=3)
                nc.vector.tensor_scalar_add(
                    ob[:, :], pj[:, :], bpo_sb[:, j : j + 1]
                )
                nc.sync.dma_start(
                    out=outT[:, j, c * CH : (c + 1) * CH], in_=ob[:, :]
                )

            def op_units(c, j):
                return [
                    lambda: oproj_mms(c, j, (0, 1)),
                    lambda: (oproj_mms(c, j, (2, 3)), oproj_finish(c, j)),
                ]

            # ---- emission plan ----
            # prologue: just enough for attention(p0, c0) to start: K and Q
            # pair-0 chunk-0 tiles and the first two V tiles.  Everything
            # else streams in as fragmented units during the s-loops.
            run_units(qk_units(4, 0))
            run_units(qk_units(0, 0))
            run_units(vp_units(0))
            run_units(vp_units(1))

            scheds = [dict() for _ in range(PAIRS)]

            def put_stream(p, slots, units):
                """Distribute units over slots round-robin, in order."""
                per = (len(units) + len(slots) - 1) // len(slots)
                it = iter(units)
                for sl in slots:
                    for _ in range(per):
                        u = next(it, None)
                        if u is None:
                            return
                        scheds[p].setdefault(sl, []).append(u)

            # pair 0 chunk 0: JIT kt chunks (deadlines s=4/8/12), the
            # remaining V tiles (vv[t] before AV(t)), and qt chunk 1.
            c0_stream = (
                qk_units(4, 1)
                + vp_units(2) + vp_units(3)
                + qk_units(4, 2)
                + vp_units(4) + vp_units(5) + vp_units(6)
                + qk_units(4, 3)
                + vp_units(7) + vp_units(8) + vp_units(9)
                + qk_units(0, 1)
                + vp_units(10) + vp_units(11) + vp_units(12)
                + vp_units(13) + vp_units(14) + vp_units(15)
            )
            put_stream(0, [(0, s) for s in range(TT)], c0_stream)
            # pair 0: qt chunks 2-3 early in c1
            put_stream(0, [(1, s) for s in (1, 2, 3, 4, 5, 6, 7, 8)],
                       qk_units(0, 2) + qk_units(0, 3))
            # pairs 0-2: next pair's Q and K tiles, 1 unit/iteration across
            # the remaining slots (K chunks before their c0 use next pair)
            for p in range(PAIRS - 1):
                units = (
                    qk_units(p + 1, 0) + qk_units(4 + p + 1, 0)
                    + qk_units(4 + p + 1, 1) + qk_units(4 + p + 1, 2)
                    + qk_units(4 + p + 1, 3)
                    + qk_units(p + 1, 1) + qk_units(p + 1, 2)
                    + qk_units(p + 1, 3)
                )
                if p == 0:
                    slots = [(1, s) for s in range(9, 16)]
                    slots += [(cc, s) for cc in (2, 3) for s in range(1, 15)]
                else:
                    slots = [(cc, s) for cc in (1, 2, 3) for s in range(1, 12)]
                put_stream(p, slots, units)
            # pair 3: output projection for chunk c streams through chunk
            # c+1 (normalize for c completes during c+1's first iterations)
            for c in range(NCHUNK - 1):
                units = []
                for j in range(8):
                    units += op_units(c, j)
                put_stream(3, [(c + 1, s) for s in range(2, 10)], units)
            tail = []
            for j in range(8):
                tail += op_units(NCHUNK - 1, j)

            for p in range(PAIRS):
                attention(p, sched=scheds[p])
            for fn in tail:
                fn()

    nc.compile()
    return nc


def rope_tables():
    """cos / sign-folded sin tables in [128 partitions, T] layout.

    Head dims are packed [evens | odds]: rows 0-31 hold x0 of pair k=row
    (sinsg = -sin), rows 32-63 hold x1 of pair k=row-32 (sinsg = +sin)."""
    k = np.arange(32).astype(np.float64)
    freqs = ROPE_BASE ** (-2.0 * k / D)  # [32]
    t = np.arange(T, dtype=np.float64)
    theta = t[None, :] * freqs[:, None]  # [32, T]
    cos64 = np.concatenate([np.cos(theta), np.cos(theta)], axis=0)
    sin64 = np.concatenate([-np.sin(theta), np.sin(theta)], axis=0)
    cos128 = np.tile(cos64, (2, 1))
    sin128 = np.tile(sin64, (2, 1))
    return _bf16(cos128), _bf16(sin128)


def pack_group_weights(w_qkv, b_qkv, w_proj, b_proj, g):
    """Per-head-group weight shards in device layout."""
    Wq, Wk, Wv = w_qkv[:, :C], w_qkv[:, C : 2 * C], w_qkv[:, 2 * C :]
    bq, bk, bv = b_qkv[:C], b_qkv[C : 2 * C], b_qkv[2 * C :]
    heads = np.arange(g * LH, (g + 1) * LH)

    # qk tiles: j 0-3 = Q pairs, 4-7 = K pairs; each tile = 2 heads x 64 dims.
    # Within each head the dims are permuted [evens | odds] so the RoPE pair
    # partner is a 32-partition block swap (QK^T invariant to shared perm).
    eo = np.concatenate([np.arange(0, D, 2), np.arange(1, D, 2)])
    qk_cols = []
    bqk_cols = []
    for src, bias in ((Wq, bq), (Wk, bk)):
        for p in range(PAIRS):
            cols = np.concatenate(
                [heads[2 * p] * D + eo, heads[2 * p + 1] * D + eo]
            )
            qk_cols.append(src[:, cols])
            bqk_cols.append(bias[cols])
    wqk_l = np.concatenate(qk_cols, axis=1)  # [C, 1024]
    wqk_dev = _bf16(wqk_l.reshape(CS, 128, 8, 128).transpose(1, 2, 0, 3))
    bqk_dev = _f32(np.stack(bqk_cols, axis=1))  # [128, 8]

    vcols = np.concatenate([np.arange(h * D, h * D + D) for h in heads])
    wv_dev = _bf16(Wv[:, vcols].reshape(CS, 128, LH * D).transpose(1, 0, 2))
    bv_dev = _f32(np.broadcast_to(bv[vcols], (128, LH * D)))

    # proj rows in attnT order: local index p*128 + e*64 + d <-> head 2p+e
    rows = np.concatenate(
        [np.arange(heads[i] * D, heads[i] * D + D) for i in range(LH)]
    )
    wpo_l = w_proj[rows, :]  # [512, 1024]
    wpo_dev = _bf16(wpo_l.reshape(PAIRS, 128, 1024).transpose(1, 0, 2))

    bpo_full = b_proj if g == 0 else np.zeros_like(b_proj)  # avoid double bias
    bpo_dev = _f32(bpo_full.reshape(8, 128).T)

    return dict(wqk=wqk_dev, bqk=bqk_dev, wv=wv_dev, bvbc=bv_dev,
                wpo=wpo_dev, bpo=bpo_dev)


def make_in_maps(x, w_qkv, b_qkv, w_proj, b_proj):
    x = np.asarray(x, dtype=np.float32)
    w_qkv = np.asarray(w_qkv, dtype=np.float32)
    b_qkv = np.asarray(b_qkv, dtype=np.float32)
    w_proj = np.asarray(w_proj, dtype=np.float32)
    b_proj = np.asarray(b_proj, dtype=np.float32)

    cos_dev, sin_dev = rope_tables()
    gw = [pack_group_weights(w_qkv, b_qkv, w_proj, b_proj, g) for g in (0, 1)]

    in_maps = []
    for core in range(NCORES):
        b, g = core // 2, core % 2
        xT_dev = _bf16(x[b].T.reshape(CS, 128, T).transpose(1, 0, 2))
        m = dict(xT=xT_dev, cosb=cos_dev, sinb=sin_dev, **gw[g])
        in_maps.append(m)
    return in_maps


_NC_CACHE = []


def get_nc():
    if not _NC_CACHE:
        _NC_CACHE.append(build_program())
    return _NC_CACHE[0]


def unshard(results):
    out = np.empty((B, T, C), dtype=np.float32)
    for b in range(B):
        acc = results[2 * b]["outT"].astype(np.float32) + results[2 * b + 1][
            "outT"
        ].astype(np.float32)
        out[b] = acc.transpose(1, 0, 2).reshape(C, T).T
    return out


def run(trace=False, **inputs):
    nc = get_nc()
    in_maps = make_in_maps(**inputs)
    res = run_bass_kernel_spmd(nc, in_maps, core_ids=list(range(NCORES)), trace=trace)
    return unshard(res.results), res


def kernel(**inputs) -> np.ndarray:
    out, _ = run(trace=False, **inputs)
    return out

